# revision 5
# baseline (speedup 1.0000x reference)
"""Single-head causal attention kernel for TRN2 (8 NeuronCores, data-parallel).

Problem: x[256,256,384] f32, Wq/Wk/Wv[384,64] f32 ->
  out = softmax(mask((x@Wq)(x@Wk)^T/8)) @ (x@Wv)  [256,256,64] f32

Sharding: batch 256 -> 8 cores x 32 batches. Weights replicated.

Per-core dataflow (all matmuls bf16, fp32 PSUM accumulate):
  1. x loaded with fp32->bf16 cast during SWDGE DMA; small leading load
     groups (1,1,2 batches) so the PE pipeline ramps early, then 4-batch
     groups; identity + first loads issued before W staging so nothing
     serializes the prologue
  2. PE-transpose x -> xT [128(C), T] chunks (projections contract C)
  3. qkv = x @ [Wq|Wk|Wv] via lhsT=xT chunks into one of 3 persistent
     193-wide SBUF tiles whose col 192 is memset to 1.0 once at setup;
     att@v then picks up row sums for free (flash-style normalize-after)
  4. PE-transpose q,k -> qT/kT [64(H), 256(T)] (scores contract H)
  5. scores^T[s,t] blocks (s0:t0,t1 | s1:t1) -> PSUM [128, 3, 128];
     exp(0.125*z) on ACT split into diag blocks (strided) + mid block
  6. causal mask: one DVE multiply on the diag blocks, tri broadcast
  7. att@v with rhs = qkv[:, s, 128:193] view (v cols + ones col)
  8. one strided DVE reciprocal + one broadcast gpsimd multiply;
     4-batch store groups with a 2+2 tail so the last store is short
"""

import numpy as np

B, T, C, H = 256, 256, 384, 64
NCORES = 8
BPC = B // NCORES  # 32 batches per core
CCH = C // 128  # 3 contraction chunks
TCH = T // 128  # 2 t-chunks

LOADG = [1, 1, 2] + [4] * 7  # batches per x-load DMA
STOREG = [4] * 7 + [2, 2]  # batches per out-store DMA

_CACHE = {}


def _build():
    import concourse.bass as bass
    import concourse.mybir as mybir
    import concourse.tile as tile
    from concourse import bacc
    from concourse.bass import broadcast_tensor_aps
    from concourse.masks import make_identity

    fp32 = mybir.dt.float32
    bf16 = mybir.dt.bfloat16

    nc = bacc.Bacc()
    x_d = nc.declare_dram_parameter("x", [BPC, T, C], fp32, isOutput=False)
    wq_d = nc.declare_dram_parameter("wq", [C, H], fp32, isOutput=False)
    wk_d = nc.declare_dram_parameter("wk", [C, H], fp32, isOutput=False)
    wv_d = nc.declare_dram_parameter("wv", [C, H], fp32, isOutput=False)
    out_d = nc.declare_dram_parameter("out", [BPC, T, H], fp32, isOutput=True)

    with tile.TileContext(nc) as tc:
        with (
            tc.tile_pool(name="singles", bufs=1) as singles,
            tc.tile_pool(name="xin", bufs=4) as xin,
            tc.tile_pool(name="work", bufs=3) as work,
            tc.tile_pool(name="vsm", bufs=3) as vsm,
            tc.tile_pool(name="outp", bufs=3) as outp,
            tc.tile_pool(name="ps_xt", bufs=2, space="PSUM") as ps_xt,
            tc.tile_pool(name="ps_qkv", bufs=2, space="PSUM") as ps_qkv,
            tc.tile_pool(name="ps_qkt", bufs=1, space="PSUM") as ps_qkt,
            tc.tile_pool(name="ps_wei", bufs=2, space="PSUM") as ps_wei,
            tc.tile_pool(name="ps_oa", bufs=1, space="PSUM") as ps_oa,
        ):
            # --- constants + early x loads (gpsimd order matters: the
            # first compute only needs ident + xb, so issue those first;
            # W staging waits on its HWDGE DMA and would stall the queue)
            ident = singles.tile([128, 128], bf16)
            make_identity(nc, ident)

            xbs = []  # (tile, start_batch, nbatch)
            starts = np.cumsum([0] + LOADG)

            def issue_load(gi):
                n = LOADG[gi]
                s = int(starts[gi])
                xb = xin.tile([128, n, TCH, C], bf16, tag=f"xb{n}")
                nc.gpsimd.dma_start(
                    out=xb,
                    in_=x_d[s:s + n].rearrange("n (c p) f -> p n c f", p=128),
                )
                xbs.append((xb, s, n))

            # W staging via HWDGE fp32 (issued on sync queue, runs in
            # parallel with the first x loads)
            wstage = singles.tile([128, CCH, 3 * H], fp32, tag="wstage")
            for wi, wd in enumerate((wq_d, wk_d, wv_d)):
                nc.sync.dma_start(
                    out=wstage[:, :, wi * H:(wi + 1) * H],
                    in_=wd.rearrange("(c p) h -> p c h", p=128),
                )

            for gi in range(3):
                issue_load(gi)

            # tri[s, t'] = 1.0 if t' >= s else 0
            tri = singles.tile([128, 128], bf16)
            nc.gpsimd.memset(tri, 1.0)
            nc.gpsimd.affine_select(
                out=tri, in_=tri,
                compare_op=mybir.AluOpType.is_ge,
                fill=0.0, base=0,
                pattern=[[1, 128]],
                channel_multiplier=-1,
            )
            # persistent qkv tiles (manual 3-deep rotation) with the ones
            # column (col 192) set once here: att@v reads [128:193] views
            qkv_tiles = []
            for i in range(3):
                q_t = singles.tile([128, TCH, 3 * H + 1], bf16, tag=f"qkv{i}")
                nc.gpsimd.memset(q_t[:, :, 3 * H], 1.0)
                qkv_tiles.append(q_t)

            wsb = singles.tile([128, CCH, 3 * H], bf16)
            for wi in range(3):
                nc.gpsimd.tensor_copy(
                    wsb[:, :, wi * H:(wi + 1) * H],
                    wstage[:, :, wi * H:(wi + 1) * H])
            # dummy PE op reading the last setup output: makes PE observe the
            # Pool tick past all constants, so per-batch PE instructions never
            # need a second (Pool) wait
            scratch_ps = ps_qkt.tile([64, 128], bf16, name="scratch",
                                     tag="qkt_ps")
            nc.tensor.transpose(scratch_ps, wsb[:, 0, 0:64], ident)

            next_load = 3
            cur = 0
            sg_iter = iter(STOREG)
            sg_n = 0
            osb = None
            sg_start = 0
            for b in range(BPC):
                if b >= xbs[cur][1] + xbs[cur][2]:
                    cur += 1
                    if next_load < len(LOADG):
                        issue_load(next_load)
                        next_load += 1
                xb, xs, xn = xbs[cur]
                bi = b - xs
                if sg_n == 0:
                    sg_n = next(sg_iter)
                    sg_start = b
                    osb = outp.tile([128, sg_n, TCH, H], fp32,
                                    tag=f"osb{sg_n}")

                # --- xT via PE transpose (bf16) ---
                xt_ps = ps_xt.tile([128, 2 * CCH, 128], bf16)
                for c in range(CCH):
                    for t in range(TCH):
                        nc.tensor.transpose(
                            xt_ps[:, c * TCH + t, :],
                            xb[:, bi, t, c * 128:(c + 1) * 128],
                            ident,
                        )
                xt = work.tile([128, 2 * CCH, 128], bf16, tag="xt")
                nc.vector.tensor_copy(xt, xt_ps)

                # --- qkv = x @ [Wq|Wk|Wv], natural [T, 192] ---
                qkv_ps = ps_qkv.tile([128, TCH, 3 * H], fp32)
                for t in range(TCH):
                    for c in range(CCH):
                        nc.tensor.matmul(
                            qkv_ps[:, t, :],
                            lhsT=xt[:, c * TCH + t, :],
                            rhs=wsb[:, c, :],
                            start=(c == 0), stop=(c == CCH - 1),
                        )
                qkv = qkv_tiles[b % 3]
                nc.scalar.copy(qkv[:, :, 0:3 * H], qkv_ps)

                # --- qT/kT via PE transpose: [64, 2, 256] (q then k) ---
                qkt_ps = ps_qkt.tile([64, 2, T], bf16)
                for qi in range(2):  # 0=q, 1=k
                    for t in range(TCH):
                        nc.tensor.transpose(
                            qkt_ps[:, qi, t * 128:(t + 1) * 128],
                            qkv[:, t, qi * H:(qi + 1) * H],
                            ident,
                        )
                qkt = work.tile([64, 2, T], bf16, tag="qkt")
                nc.vector.tensor_copy(qkt, qkt_ps)

                # --- scores^T blocks: [128, 3, 128] = (s0t0, s0t1, s1t1) ---
                wei_ps = ps_wei.tile([128, 3, 128], fp32)
                nc.tensor.matmul(
                    wei_ps[:, 0:2, :],
                    lhsT=qkt[:, 1, 0:128], rhs=qkt[:, 0, :],
                    start=True, stop=True,
                )
                nc.tensor.matmul(
                    wei_ps[:, 2, :],
                    lhsT=qkt[:, 1, 128:256], rhs=qkt[:, 0, 128:256],
                    start=True, stop=True,
                )
                # exp(z/8) on ACT: diag blocks (strided) + mid block
                mdiag = work.tile([128, 2, 128], bf16, tag="mdiag")
                mmid = work.tile([128, 128], bf16, tag="mmid")
                nc.scalar.activation(
                    out=mdiag, in_=wei_ps[:, 0::2, :],
                    func=mybir.ActivationFunctionType.Exp,
                    scale=float(H) ** -0.5,
                )
                nc.scalar.activation(
                    out=mmid, in_=wei_ps[:, 1, :],
                    func=mybir.ActivationFunctionType.Exp,
                    scale=float(H) ** -0.5,
                )
                # causal mask on the diag blocks, tri broadcast over dim 1
                m_ap, t_ap = broadcast_tensor_aps(mdiag[:, :, :],
                                                  tri[:, None, :])
                nc.vector.tensor_mul(mdiag, m_ap, t_ap)

                # --- att @ v_aug -> out_aug [T, 65] per t-chunk ---
                # rhs = qkv[:, s, 128:193] view: v cols + ones col
                oa_ps = ps_oa.tile([128, 2, H + 1], fp32)
                nc.tensor.matmul(
                    oa_ps[:, 0, :], lhsT=mdiag[:, 0, :],
                    rhs=qkv[:, 0, 2 * H:3 * H + 1],
                    start=True, stop=True,
                )
                nc.tensor.matmul(
                    oa_ps[:, 1, :], lhsT=mmid,
                    rhs=qkv[:, 0, 2 * H:3 * H + 1],
                    start=True, stop=False,
                )
                nc.tensor.matmul(
                    oa_ps[:, 1, :], lhsT=mdiag[:, 1, :],
                    rhs=qkv[:, 1, 2 * H:3 * H + 1],
                    start=False, stop=True,
                )

                # --- normalize (recip + broadcast multiply on DVE) ---
                rec = vsm.tile([128, 2], fp32, tag="rec")
                nc.vector.reciprocal(rec, oa_ps[:, :, H])
                o_ap, r_ap = broadcast_tensor_aps(
                    oa_ps[:, :, 0:H], rec[:, :, None])
                nc.vector.tensor_mul(osb[:, b - sg_start], o_ap, r_ap)

                sg_n -= 1
                if sg_n == 0:
                    n = osb.shape[1]
                    nc.sync.dma_start(
                        out=out_d[sg_start:sg_start + n].rearrange(
                            "n (c p) h -> p n c h", p=128),
                        in_=osb,
                    )
    nc.compile()
    return nc


def _get_nc():
    if "nc" not in _CACHE:
        _CACHE["nc"] = _build()
    return _CACHE["nc"]


def kernel(x, Wq, Wk, Wv):
    from concourse.bass_utils import run_bass_kernel_spmd

    x = np.ascontiguousarray(np.asarray(x, dtype=np.float32))
    Wq = np.ascontiguousarray(np.asarray(Wq, dtype=np.float32))
    Wk = np.ascontiguousarray(np.asarray(Wk, dtype=np.float32))
    Wv = np.ascontiguousarray(np.asarray(Wv, dtype=np.float32))

    nc = _get_nc()
    in_maps = [
        {"x": x[i * BPC:(i + 1) * BPC], "wq": Wq, "wk": Wk, "wv": Wv}
        for i in range(NCORES)
    ]
    res = run_bass_kernel_spmd(nc, in_maps, list(range(NCORES)))
    return np.concatenate([res.results[i]["out"] for i in range(NCORES)], axis=0)
